# revision 7
# baseline (speedup 1.0000x reference)
"""EyesMouthLoss Trainium2 kernel.

loss = mean(|pred-target| * (1 + 299*clip(eye_mask+mouth_mask, 0, 1)))

Sharding: pure data-parallel over B=16 -> 2 batches per core on 8 cores.
Host sums the 8 per-core partial scalars (the final all-reduce).

The masks depend only on `landmarks` (tiny: 16x68x2 ints), so the host
precomputes the per-pixel weight w = 1 + 299*clip(eye+mouth, 0, 1) and
ships it per core as bf16 (1 MB next to the 12.6 MB/core of fp32
pred/target).

All loads ride HWDGE (SP engine): HWDGE issues start ~5 us earlier than
SWDGE and avoid the SWDGE descriptor-ring port contention that makes
SDMA engines 7/15 straggle (which delays every SWDGE completion
semaphore by a growing ~15%).  pred/target land in fp32 staging rings;
the fp32->bf16 cast happens inside the first compute op:

    d   = pred - target     DVE tensor_tensor, fp32 ins -> bf16 out
    a   = |d|               ACT Abs (bf16), fp32 accum_out (row |d| sum)
    g   = a * w             DVE scalar_tensor_tensor, w broadcast over
                            channels, fp32 accum_out = weighted row-sum

The [128, 16] fp32 accumulator tile is the only output; the host
applies the final 1/N while summing the 8 per-core partials.
"""

import sys

sys.path.insert(0, "/opt/trn_rl_repo")

from contextlib import ExitStack

import ml_dtypes
import numpy as np

import concourse.bass as bass
import concourse.tile as tile
from concourse import bacc, mybir
from concourse.bass_utils import run_bass_kernel_spmd

B, C, H, W = 16, 3, 512, 512
NCORES = 8
BPC = B // NCORES  # batches per core
NCHUNK = 4  # 512 rows = 4 x 128 partitions
NSTAGE = 4  # fp32 staging ring depth (units in flight)
RADIUS = 15.0
HALF = 14  # region strictly zero for |dx| >= 15
EYE = (36, 48)
MOUTH = (48, 68)
WEIGHT = 300.0
NTOT = float(B * C * H * W)
FP32 = mybir.dt.float32
BF16 = mybir.dt.bfloat16
Alu = mybir.AluOpType
Act = mybir.ActivationFunctionType

_STENCIL = None


def _stencil():
    global _STENCIL
    if _STENCIL is None:
        d = np.arange(2 * HALF + 1, dtype=np.float32) - HALF
        r = np.sqrt(d[:, None] ** 2 + d[None, :] ** 2)
        _STENCIL = np.clip(1.0 - r / RADIUS, 0.0, 1.0).astype(np.float32)
    return _STENCIL


def _weights(landmarks):
    """w[b,y,x] = 1 + 299*clip(eye+mouth, 0, 1), computed on host."""
    st = _stencil()
    w = np.empty((B, H, W), np.float32)
    for b in range(B):
        fields = np.zeros((2, H, W), np.float32)
        for field, (lo, hi) in zip(fields, (EYE, MOUTH)):
            for cx, cy in landmarks[b, lo:hi]:
                cx = int(min(max(int(cx), 0), W - 1))
                cy = int(min(max(int(cy), 0), H - 1))
                y0, y1 = max(0, cy - HALF), min(H - 1, cy + HALF)
                x0, x1 = max(0, cx - HALF), min(W - 1, cx + HALF)
                sy0, sx0 = y0 - (cy - HALF), x0 - (cx - HALF)
                np.maximum(
                    field[y0 : y1 + 1, x0 : x1 + 1],
                    st[sy0 : sy0 + y1 - y0 + 1, sx0 : sx0 + x1 - x0 + 1],
                    out=field[y0 : y1 + 1, x0 : x1 + 1],
                )
        w[b] = 1.0 + (WEIGHT - 1.0) * np.minimum(fields[0] + fields[1], 1.0)
    return w


def _build():
    """Build the SPMD Bass program (shared by all cores; data-parallel)."""
    nc = bacc.Bacc(None)
    pred_p = nc.declare_dram_parameter("pred", [BPC, C, H, W], FP32, isOutput=False)
    targ_p = nc.declare_dram_parameter("targ", [BPC, C, H, W], FP32, isOutput=False)
    wgt_p = nc.declare_dram_parameter(
        "wgt", [BPC, NCHUNK, 128, W], BF16, isOutput=False
    )
    out_p = nc.declare_dram_parameter(
        "out", [128, 2 * BPC * NCHUNK], FP32, isOutput=True
    )

    with tile.TileContext(nc) as tc, ExitStack() as ctx:
        stat_pool = ctx.enter_context(tc.tile_pool(name="stat", bufs=2))
        load_pool = ctx.enter_context(tc.tile_pool(name="load", bufs=2))

        units = [(bi, k) for bi in range(BPC) for k in range(NCHUNK)]
        nu = len(units)
        rs = stat_pool.tile([128, 2 * nu], FP32)  # [abs sums | weighted sums]

        w_t = load_pool.tile([128, BPC, NCHUNK, W], BF16, tag="w_t")
        # fp32 staging rings (NSTAGE units in flight) + bf16 work tiles
        stage_p = [
            load_pool.tile([128, C, W], FP32, tag="sp", name=f"sp{s}")
            for s in range(NSTAGE)
        ]
        stage_t = [
            load_pool.tile([128, C, W], FP32, tag="st", name=f"st{s}")
            for s in range(NSTAGE)
        ]
        d_t = [
            load_pool.tile([128, C, W], BF16, tag="d", name=f"d{s}")
            for s in range(NSTAGE)
        ]
        a_t = [
            load_pool.tile([128, C, W], BF16, tag="a", name=f"a{s}")
            for s in range(NSTAGE)
        ]

        # w loads first on the same HWDGE FIFO
        for bi in range(BPC):
            nc.sync.dma_start(
                w_t[:, bi, :, :],
                wgt_p[bi].rearrange("k p x -> p k x"),
            )

        def load(u):
            bi, k = units[u]
            s = u % NSTAGE
            rows = slice(128 * k, 128 * (k + 1))
            nc.sync.dma_start(
                stage_p[s][:], pred_p[bi, :, rows, :].rearrange("c p x -> p c x")
            )
            nc.sync.dma_start(
                stage_t[s][:], targ_p[bi, :, rows, :].rearrange("c p x -> p c x")
            )

        def sub(u):
            s = u % NSTAGE
            nc.vector.tensor_tensor(
                d_t[s][:], stage_p[s][:], stage_t[s][:], op=Alu.subtract
            )

        def abs_(u):
            s = u % NSTAGE
            nc.scalar.activation(
                a_t[s][:], d_t[s][:], Act.Abs, accum_out=rs[:, u : u + 1]
            )

        def stt(u):
            bi, k = units[u]
            s = u % NSTAGE
            wb = (
                w_t[:, bi, k, :]
                .broadcast_to([128, W, C])
                .rearrange("p x c -> p c x")
            )
            nc.vector.scalar_tensor_tensor(
                d_t[s][:], a_t[s][:], 1.0, wb,
                op0=Alu.mult, op1=Alu.mult,
                accum_out=rs[:, nu + u : nu + u + 1],
            )

        # prime the load pipeline, then stream: the DVE queue is
        # [sub0, sub1, stt0, sub2, stt1, ...] so its head never waits on
        # the cross-engine ABS
        for u in range(min(NSTAGE, nu)):
            load(u)
        sub(0)
        abs_(0)
        for u in range(1, nu):
            if u + NSTAGE - 1 < nu:
                load(u + NSTAGE - 1)
            sub(u)
            abs_(u)
            stt(u - 1)
        stt(nu - 1)

        nc.sync.dma_start(out_p[:, :], rs[:])

    return nc


def run(inputs, trace=False):
    pred = np.ascontiguousarray(inputs["pred"], dtype=np.float32)
    targ = np.ascontiguousarray(inputs["target"], dtype=np.float32)
    lms = np.asarray(inputs["landmarks"])
    assert pred.shape == (B, C, H, W) and targ.shape == (B, C, H, W)

    w = _weights(lms).reshape(B, NCHUNK, 128, W).astype(ml_dtypes.bfloat16)

    nc = _build()
    nc.finalize()
    in_maps = [
        {
            "pred": pred[i * BPC : (i + 1) * BPC],
            "targ": targ[i * BPC : (i + 1) * BPC],
            "wgt": w[i * BPC : (i + 1) * BPC],
        }
        for i in range(NCORES)
    ]
    res = run_bass_kernel_spmd(nc, in_maps, list(range(NCORES)), trace=trace)
    nu = BPC * NCHUNK
    total = 0.0
    for i in range(NCORES):
        part = res.results[i]["out"].astype(np.float64)
        total += part[:, nu:].sum()
    return np.float32(total / NTOT), res


def kernel(pred, target, landmarks):
    out, _ = run({"pred": pred, "target": target, "landmarks": landmarks})
    return out
